# revision 28
# baseline (speedup 1.0000x reference)
"""Trainium2 Bass kernel: bidirectional conv-BN-relu message passing over H.

Reference semantics (per batch item, channels C, scan over H):
  forward:  new[0] = x[0];   new[h] = relu(bn(conv(new[h-1]))) + x[h]
  backward: out[H-1] = new[H-1]; out[h] = relu(bn(conv(out[h+1]))) + new[h]
conv = 1D conv along W, kernel 9, pad 4, C->C channels; BN (eval mode)
is a per-channel affine y*s + t.

Strategy: data-parallel over B across 8 cores (2 batch items per core).
Each conv step = 9 shifted-window matmuls accumulated in PSUM
(lhsT = per-tap [I,O] weights, rhs = padded state slice windows).

Matmul operands are bf16: fp32 weights disable the PE's fast-weight-load
path (LDWEIGHTS at ~225ns/matmul dominates the ~109ns N=256 stream time),
while bf16 keeps the conv recurrence within ~7e-3 relative error (fp64
sim) because PSUM accumulation and the x-carry adds stay fp32.

The affine+relu+carry tail is algebraically folded into ONE DVE op:
 - the BN scale s is folded into the weights host-side (W' = s[o]*W),
 - the state is stored shifted: n = new - r, with pad columns = -r,
   where r solves r = t + M r (M[o,i] = sum_k W'[o,i,k]); then
   conv(n-padded) = conv(new) - (r - t), and the update collapses to
     n_h = max(psum, -r) + carry          (single scalar_tensor_tensor)
   r is added back to the gathered output on the host.
The backward carry is new_h = n_h + r, staged per step on the (otherwise
idle) Activation engine so the tensor engine runs nothing but the 9-tap
conv groups.

Startup/drain engineering: the h=0 padded state slice is precomputed on
the host in bf16 and lands in one small DMA; params lead the sync queue;
the weight DMA is split so the first taps arrive early; a short burst of
dummy matmuls during the DMA wait lifts the PE HAM clock-gate (1.2 ->
2.4 GHz takes ~3.4us of busyness) before the real stream starts. Input
slices arrive in growing batches; outputs leave pad-inclusive (contiguous
528B*OB runs instead of 512B packets) on two queues, sliced on the host.
"""

import os
from contextlib import ExitStack

import numpy as np
import ml_dtypes

import bass_rust
import concourse.bass as bass
import concourse.tile as tile
from concourse import mybir
from concourse.bass_utils import run_bass_kernel_spmd

B, C, H, W = 16, 128, 64, 256
K, PAD = 9, 4
NCORES = 8
BPC = B // NCORES  # batch items per core
WP = W + 2 * PAD
EPS = 1e-5
OB = 4  # output-slice DMA batch
NWARM = 11  # dummy matmuls to lift the HAM clock-gate, sized to end about
# when the first weight/state DMAs land (~2.5us after the PE queue opens)

F32 = mybir.dt.float32
BF16 = mybir.dt.bfloat16
NP_BF16 = ml_dtypes.bfloat16
IDENT = mybir.ActivationFunctionType.Identity

_NC_CACHE: dict = {}
LAST_RESULTS = None  # stashed BassKernelResults for test.py introspection


def _xbounds(h_dim):
    """Input-batch spans for h>=1: small leading batches so the first conv
    steps aren't gated on a bulk transfer, then steady groups of 4."""
    bounds, sizes, lo = [], [1, 2, 4], 1
    while lo < h_dim:
        sz = sizes[0] if sizes else 4
        if sizes:
            sizes = sizes[1:]
        bounds.append((lo, min(lo + sz, h_dim)))
        lo += sz
    return bounds


def _build_nc(bpc=BPC, h_dim=H, w_dim=W):
    wp = w_dim + 2 * PAD
    nc = bass.Bass()
    x_d = nc.dram_tensor("x", [bpc, C, h_dim, w_dim], F32, kind="ExternalInput")
    n0_d = nc.dram_tensor("n0", [bpc, C, wp], BF16, kind="ExternalInput")
    w_d = nc.dram_tensor("w", [C, K, C], BF16, kind="ExternalInput")
    pr_d = nc.dram_tensor("pr", [C, 2], F32, kind="ExternalInput")  # [-r, +r]
    o_d = nc.dram_tensor("o", [bpc, C, h_dim, wp], BF16, kind="ExternalOutput")

    add = mybir.AluOpType.add
    mx = mybir.AluOpType.max

    xb_list = _xbounds(h_dim)
    xb_of_h = {}
    for i, (lo, hi) in enumerate(xb_list):
        for h in range(lo, hi):
            xb_of_h[h] = (i, lo, hi)

    with ExitStack() as ctx:
        tc = ctx.enter_context(tile.TileContext(nc))
        singles = ctx.enter_context(tc.tile_pool(name="singles", bufs=1))
        big = ctx.enter_context(tc.tile_pool(name="big", bufs=1))
        xs_pool = ctx.enter_context(tc.tile_pool(name="xs", bufs=6))
        nr_pool = ctx.enter_context(tc.tile_pool(name="nr", bufs=4))
        pp = ctx.enter_context(tc.tile_pool(name="pp", bufs=8, space="PSUM"))

        # --- DMA ordering: the h=0 state slices gate the first conv group,
        # so they lead the sync queue; the tiny (per-4B-packet, slow) param
        # vectors follow; the (large) weight tile rides the scalar queue in
        # two chunks so the first taps arrive before the rest.
        new = []
        for c in range(bpc):
            nt = big.tile([C, h_dim, wp], BF16, tag=f"new{c}", name=f"new{c}")
            nc.sync.dma_start(out=nt[:, 0, :], in_=n0_d[c])
            new.append(nt)
        prt = singles.tile([C, 2], F32, tag="prt", name="prt")
        nc.sync.dma_start(out=prt, in_=pr_d[:, :])
        bt = prt[:, 0:1]
        rt = prt[:, 1:2]

        wt = singles.tile([C, K, C], BF16, tag="wt", name="wt")
        nc.scalar.dma_start(out=wt[:, 0:2, :], in_=w_d[:, 0:2, :])
        nc.scalar.dma_start(out=wt[:, 2:K, :], in_=w_d[:, 2:K, :])
        wr = [wt[:, k, :] for k in range(K)]

        # --- HAM warmup: dummy matmuls on a zeroed tile keep the PE busy
        # while the DMAs land, releasing the activity clock-gate.
        dummy = singles.tile([C, w_dim], BF16, tag="dummy", name="dummy")
        nc.gpsimd.memset(dummy, 0.0)
        wm = pp.tile([C, w_dim], F32, tag="pt", name="wm", bufs=4)
        for _ in range(NWARM):
            nc.tensor.matmul(wm, dummy[:, 0:C], dummy, start=True, stop=True)

        # Pad columns for h>=1 hold the per-channel shift -r (not zero):
        # broadcast-add -r onto a zeroed fp32 tile, converting to bf16 on
        # write. (h=0 pads arrive pre-filled in the n0 DMA.)
        zp = singles.tile([C, h_dim - 1, 2 * PAD], F32, tag="zp", name="zp")
        nc.vector.memset(zp, 0.0)
        for c in range(bpc):
            nc.vector.tensor_scalar(
                out=new[c][:, 1:h_dim, 0:PAD], in0=zp[:, :, 0:PAD],
                scalar1=bt, scalar2=None, op0=add,
            )
            nc.vector.tensor_scalar(
                out=new[c][:, 1:h_dim, PAD + w_dim : wp],
                in0=zp[:, :, PAD : 2 * PAD],
                scalar1=bt, scalar2=None, op0=add,
            )

        def conv_group(src_slice, pt):
            for k in range(K):
                nc.tensor.matmul(
                    pt,
                    wr[k],
                    src_slice[:, k : k + w_dim],
                    start=(k == 0),
                    stop=(k == K - 1),
                )

        # Forward scan over H (both chains interleaved per h).
        xtiles: list[dict[int, object]] = [dict() for _ in range(bpc)]
        for h in range(1, h_dim):
            bi, lo, hi = xb_of_h[h]
            if h == lo:
                for c in range(bpc):
                    xb = xs_pool.tile([C, 4, w_dim], F32, tag="xb", name="xb")
                    nc.sync.dma_start(
                        out=xb[:, 0 : hi - lo, :], in_=x_d[c][:, lo:hi, :]
                    )
                    xtiles[c][bi] = xb
            for c in range(bpc):
                pt = pp.tile([C, w_dim], F32, tag="pt", name="pt", bufs=4)
                conv_group(new[c][:, h - 1, :], pt)
                nc.vector.scalar_tensor_tensor(
                    out=new[c][:, h, PAD : PAD + w_dim],
                    in0=pt,
                    scalar=bt,
                    in1=xtiles[c][bi][:, h - lo, :],
                    op0=mx,
                    op1=add,
                )

        # Backward scan; out[h] overwrites new[h] in place, then streams out
        # pad-inclusive in batches of OB slices (contiguous runs DMA much
        # faster than pad-strided 512B packets), one queue per chain.
        oq = [nc.scalar, nc.sync]
        for h in range(h_dim - 2, 0, -1):
            for c in range(bpc):
                # Stage the true backward carry new_h = n_h + r on the ACT
                # engine (reads the forward state before it's overwritten).
                nr = nr_pool.tile([C, w_dim], F32, tag="nr", name="nr")
                nc.scalar.activation(
                    out=nr, in_=new[c][:, h, PAD : PAD + w_dim],
                    func=IDENT, bias=rt, scale=1.0,
                )
                pt = pp.tile([C, w_dim], F32, tag="pt", name="pt", bufs=4)
                conv_group(new[c][:, h + 1, :], pt)
                nc.vector.scalar_tensor_tensor(
                    out=new[c][:, h, PAD : PAD + w_dim],
                    in0=pt,
                    scalar=bt,
                    in1=nr,
                    op0=mx,
                    op1=add,
                )
            if h == 2:
                # Split the final OB-batch so the very last transfer (which
                # gates the drain) is only 2 slices.
                for c in range(bpc):
                    oq[c % 2].dma_start(
                        out=o_d[c][:, 2:4, :], in_=new[c][:, 2:4, :]
                    )
            elif h == 1:
                for c in range(bpc):
                    oq[c % 2].dma_start(
                        out=o_d[c][:, 1:2, :], in_=new[c][:, 1:2, :]
                    )
            elif h % OB == 0:
                hi = min(h + OB, h_dim)
                for c in range(bpc):
                    oq[c % 2].dma_start(
                        out=o_d[c][:, h:hi, :], in_=new[c][:, h:hi, :]
                    )

        # Final step (h=0) in two half-width PSUM groups per chain so the
        # very last DVE op and output transfer are half-sized — they sit on
        # the kernel's drain critical path.
        hw2 = w_dim // 2
        for c in range(bpc):
            nr = nr_pool.tile([C, w_dim], F32, tag="nr", name="nr")
            nc.scalar.activation(
                out=nr, in_=new[c][:, 0, PAD : PAD + w_dim],
                func=IDENT, bias=rt, scale=1.0,
            )
            for half in range(2):
                lo = half * hw2
                pt = pp.tile([C, hw2], F32, tag="pth", name="pth", bufs=2)
                for k in range(K):
                    nc.tensor.matmul(
                        pt,
                        wr[k],
                        new[c][:, 1, lo + k : lo + k + hw2],
                        start=(k == 0),
                        stop=(k == K - 1),
                    )
                nc.vector.scalar_tensor_tensor(
                    out=new[c][:, 0, PAD + lo : PAD + lo + hw2],
                    in0=pt,
                    scalar=bt,
                    in1=nr[:, lo : lo + hw2],
                    op0=mx,
                    op1=add,
                )
                # Pad-inclusive half-slice: left half carries the left pads,
                # right half the right pads.
                if half == 0:
                    oq[c % 2].dma_start(
                        out=o_d[c][:, 0, 0 : PAD + hw2],
                        in_=new[c][:, 0, 0 : PAD + hw2],
                    )
                else:
                    oq[c % 2].dma_start(
                        out=o_d[c][:, 0, PAD + hw2 : wp],
                        in_=new[c][:, 0, PAD + hw2 : wp],
                    )

    # TRN2 caps most instructions at one semaphore wait (matmuls lower to an
    # LDWEIGHTS struct with a single wait slot); split any excess onto
    # EventSemaphore instructions like bacc does.
    bass_rust.generate_event_semaphores(nc)
    return nc


def _get_nc():
    key = (BPC, H, W)
    if key not in _NC_CACHE:
        _NC_CACHE[key] = _build_nc()
    return _NC_CACHE[key]


def _prep_params(conv_w, gamma, beta, run_mean, run_var):
    """Fold BN scale into the weights and solve the state shift r.

    Returns (w_t [I,K,O] bf16 with s folded, -r as [C,1] f32, r as [C] f64).
    """
    s = gamma.astype(np.float64) / np.sqrt(run_var.astype(np.float64) + EPS)
    t = beta.astype(np.float64) - run_mean.astype(np.float64) * s
    w_s = s[:, None, None] * conv_w.astype(np.float64)  # [O,I,K]
    m = w_s.sum(axis=2)  # [O,I]
    r = np.linalg.solve(np.eye(C) - m, t)
    w_t = np.ascontiguousarray(w_s.transpose(1, 2, 0)).astype(NP_BF16)
    rneg = (-r).astype(np.float32).reshape(C, 1)
    return w_t, rneg, r


def kernel(inputs, conv_w, gamma, beta, run_mean, run_var):
    global LAST_RESULTS
    conv_w, gamma, beta, run_mean, run_var = (
        np.asarray(a) for a in (conv_w, gamma, beta, run_mean, run_var)
    )
    w_t, rneg, r = _prep_params(conv_w, gamma, beta, run_mean, run_var)
    x = np.ascontiguousarray(np.asarray(inputs, dtype=np.float32))  # [B,C,H,W]
    rf = r.astype(np.float32)
    # Padded h=0 state slice in bf16: pads = -r, interior = x[:, :, 0] - r.
    n0 = np.empty((B, C, WP), np.float32)
    n0[:, :, :] = -rf[None, :, None]
    n0[:, :, PAD : PAD + W] = x[:, :, 0] - rf[None, :, None]
    n0 = n0.astype(NP_BF16)
    pr = np.ascontiguousarray(np.concatenate([rneg, -rneg], axis=1))
    in_maps = [
        dict(
            x=x[c * BPC : (c + 1) * BPC],
            n0=n0[c * BPC : (c + 1) * BPC],
            w=w_t,
            pr=pr,
        )
        for c in range(NCORES)
    ]
    nc = _get_nc()
    trace = os.environ.get("KERNEL_TRACE", "0") == "1"
    res = run_bass_kernel_spmd(
        nc, in_maps, core_ids=list(range(NCORES)), trace=trace
    )
    LAST_RESULTS = res
    out = np.concatenate(
        [np.asarray(res.results[c]["o"]) for c in range(NCORES)], axis=0
    )[:, :, :, PAD : PAD + W].astype(np.float32)
    return out + rf[None, :, None, None]  # back to out-space
